# revision 19
# baseline (speedup 1.0000x reference)
"""Trainium2 Bass kernel for causal sliding-window self-attention.

Shapes (hardcoded): B=2, T=2048, NH=12, HD=128, HIDDEN=1536, window=1024.

Sharding: 8 cores; core c handles batch b=c//4 and heads [3*(c%4), 3*(c%4)+3).
Each core computes q/k/v projections for its 3 heads (contraction over the
full hidden dim), RoPE + RMS-norm, block-sparse attention (query block i
attends key blocks [i-8, i]), and a partial output projection. The host sums
the 4 partial projections per batch element. No collectives.

Layout strategy per head:
  - project q,k,v into [T, d] (token-major) so RoPE/RMS-norm reductions are
    free-dim reductions; PE-transpose q,k to [d, T] (fp16)
  - scores computed transposed: s_t[c, r] = k_tile.T @ q_pair  (so p @ v needs
    no transpose of p);  q pre-scaled by rms_q, k pre-scaled by rms_k/sqrt(HD)
  - softmax denominator: a ones-column appended to v accumulates sum(p) in the
    same PSUM tile as p@v
  - exp computed as exp(s - 3) (softmax shift-invariant) to keep fp16 p finite

Engine balance: QKV matmuls use 3-pass hi/lo fp8 DoubleRow (0.75x the PE cost
of fp16).  DVE handles only PSUM-reading elementwise work (rope muls, p@v
normalize, PSUM->SBUF copies); rope adds, rms Newton-rsqrt, rq scaling and
mask multiplies run on Pool; squares/exp/v-copy on Act.  Scores matmuls are
trimmed to exactly the query columns that need each key block.  PSUM: 8 banks
= psq+psk (2) + psv double-buffered (2) + transpose staging x2 (2) +
scores/p@v/out-proj shared 2-slot rotation (2).
"""

import sys
import os

sys.path.insert(0, "/opt/trn_rl_repo")

import numpy as np
from contextlib import ExitStack

import concourse.bass as bass
import concourse.bacc as bacc
import concourse.tile as tile
from concourse import mybir
from concourse.bass_utils import run_bass_kernel_spmd

F32 = mybir.dt.float32
F16 = mybir.dt.float16
AF = mybir.ActivationFunctionType
MUL = mybir.AluOpType.mult
ADD = mybir.AluOpType.add

B, T, NH, HD = 2, 2048, 12, 128
HIDDEN = NH * HD
EPS = 1.1920928955078125e-07
NB = T // 128        # 16 token blocks
KT = HIDDEN // 128   # 12 contraction tiles
WB = 8               # window in blocks (1024/128)
NHC = 3              # heads per core
EXP_SHIFT = -3.0     # exp(s + EXP_SHIFT); cancels in softmax, keeps fp16 finite

_cached_nc = None
_CFG = {}


def _window(i):
    return list(range(max(0, i - WB), i + 1))


def _build(stages="ABC", cfg=None):
    cfg = dict(cfg) if cfg else {}
    cfg.setdefault("pipe", 1)
    cfg.setdefault("vdve", 0)    # vext copies: 0=Act, 1=DVE
    cfg.setdefault("osbdve", 0)  # out-proj copies: alternate DVE/Act
    cfg.setdefault("padd", 0)    # rope adds on Pool
    cfg.setdefault("rqpool", 1)  # rq scaling on Pool
    cfg.setdefault("newti", 2)   # Newton iterations (seed is minimax-fit)
    cfg.setdefault("psq", 1)     # psq bufs
    cfg.setdefault("psk", 1)     # psk bufs
    cfg.setdefault("psv2", 1)    # double-buffer psv
    cfg.setdefault("tp2", 1)     # double-buffer transpose staging
    cfg.setdefault("odma", 0)    # (dead: dma_start cannot read PSUM)
    nc = bacc.Bacc("TRN2", target_bir_lowering=False, debug=False, num_devices=8)

    F8 = mybir.dt.float8e4
    x8h = nc.dram_tensor("x8h", [HIDDEN, T], F8, kind="ExternalInput")
    x8l = nc.dram_tensor("x8l", [HIDDEN, T], F8, kind="ExternalInput")
    wq8h = nc.dram_tensor("wq8h", [HIDDEN, NHC * HD], F8, kind="ExternalInput")
    wq8l = nc.dram_tensor("wq8l", [HIDDEN, NHC * HD], F8, kind="ExternalInput")
    wk8h = nc.dram_tensor("wk8h", [HIDDEN, NHC * HD], F8, kind="ExternalInput")
    wk8l = nc.dram_tensor("wk8l", [HIDDEN, NHC * HD], F8, kind="ExternalInput")
    wv8h = nc.dram_tensor("wv8h", [HIDDEN, NHC * HD], F8, kind="ExternalInput")
    wv8l = nc.dram_tensor("wv8l", [HIDDEN, NHC * HD], F8, kind="ExternalInput")
    wp16 = nc.dram_tensor("wp16", [NHC * HD, HIDDEN], F16, kind="ExternalInput")
    cos3 = nc.dram_tensor("cos3", [T, NHC * 64], F32, kind="ExternalInput")
    sin3 = nc.dram_tensor("sin3", [T, NHC * 64], F32, kind="ExternalInput")
    mdiag = nc.dram_tensor("mdiag", [128, 128], F16, kind="ExternalInput")
    medge = nc.dram_tensor("medge", [128, 128], F16, kind="ExternalInput")
    ident = nc.dram_tensor("ident", [128, 128], F16, kind="ExternalInput")
    outp = nc.dram_tensor(
        "outp", [T, HIDDEN], F32 if cfg["odma"] else F16, kind="ExternalOutput"
    )

    with tile.TileContext(nc) as tc:
        with ExitStack() as ctx:
            const = ctx.enter_context(tc.tile_pool(name="const", bufs=1))
            persist = ctx.enter_context(tc.tile_pool(name="persist", bufs=1))

            # --- constants / weights -------------------------------------
            wqth = const.tile([128, KT, NHC * HD], F8)
            wqtl = const.tile([128, KT, NHC * HD], F8)
            wkth = const.tile([128, KT, NHC * HD], F8)
            wktl = const.tile([128, KT, NHC * HD], F8)
            wvth = const.tile([128, KT, NHC * HD], F8)
            wvtl = const.tile([128, KT, NHC * HD], F8)
            for dst_, src_ in (
                (wqth, wq8h), (wqtl, wq8l), (wkth, wk8h),
                (wktl, wk8l), (wvth, wv8h), (wvtl, wv8l),
            ):
                nc.sync.dma_start(
                    dst_[:], src_.ap().rearrange("(k p) n -> p k n", p=128)
                )
            wpt = const.tile([128, NHC, HIDDEN], F16)
            nc.sync.dma_start(wpt[:], wp16.ap().rearrange("(k p) n -> p k n", p=128))
            cost = const.tile([128, NB, NHC, 64], F32)
            sint = const.tile([128, NB, NHC, 64], F32)
            nc.sync.dma_start(
                cost[:], cos3.ap().rearrange("(m p) (h c) -> p m h c", p=128, h=NHC)
            )
            nc.sync.dma_start(
                sint[:], sin3.ap().rearrange("(m p) (h c) -> p m h c", p=128, h=NHC)
            )
            mdg = const.tile([128, 128], F16)
            medg = const.tile([128, 128], F16)
            idt = const.tile([128, 128], F16)
            nc.sync.dma_start(mdg[:], mdiag.ap())
            nc.sync.dma_start(medg[:], medge.ap())
            nc.sync.dma_start(idt[:], ident.ap())
            biast = const.tile([128, 4], F32)
            nc.vector.memset(biast[:, 0:1], EPS)
            nc.vector.memset(biast[:, 1:2], HD * EPS)
            nc.vector.memset(biast[:, 2:3], EXP_SHIFT)
            b_shift = biast[:, 2:3]

            # --- persistent per-head tensors -----------------------------
            qt = persist.tile([128, NHC, T], F16)   # q^T  [d, t] per head
            kt = persist.tile([128, NHC, T], F16)   # k^T  [d, t] per head
            yt = persist.tile([128, NHC, T], F16)   # y^T  [d, t] per head
            vext = persist.tile([128, NHC, NB, 132], F16)  # v tiles + ones col
            nc.gpsimd.memset(vext[:], 1.0)  # col 128 stays 1.0; 0:128 overwritten

            x8hr = x8h.ap().rearrange("(k p) t -> p k t", p=128)
            x8lr = x8l.ap().rearrange("(k p) t -> p k t", p=128)
            # constants for folding the 64x weight pre-scale and the
            # 1/sqrt(HD) score scale (k side) into the rq write
            WS = 64.0
            C_Q = 1.0 / WS
            C_K = 1.0 / (WS * np.sqrt(128.0))
            SQ_SCALE = 1.0 / (WS * np.sqrt(128.0))  # (x*s)^2 summed -> mean(q^2)

            # pools for the fused loop
            xp = ctx.enter_context(tc.tile_pool(name="xp", bufs=cfg.get("xp", 4)))
            rp = ctx.enter_context(tc.tile_pool(name="rp", bufs=cfg.get("rp", 3)))
            pp = ctx.enter_context(tc.tile_pool(name="pp", bufs=cfg.get("pp", 17)))
            yp = ctx.enter_context(tc.tile_pool(name="yp", bufs=cfg.get("yp", 3)))
            op_sb = ctx.enter_context(tc.tile_pool(name="opsb", bufs=cfg.get("osb", 3)))
            # PSUM: psq+psk (2 banks) + psv x bufs (1-2) + tp staging x bufs
            # (1-2) + shared sps/ops/po rotation (2)
            psA = ctx.enter_context(
                tc.tile_pool(name="psA", bufs=1, space="PSUM")
            )
            tpps = ctx.enter_context(
                tc.tile_pool(name="tpps", bufs=2 if cfg["tp2"] else 1, space="PSUM")
            )
            spsum = ctx.enter_context(
                tc.tile_pool(name="spsum", bufs=cfg.get("sps", 2), space="PSUM")
            )

            DR = mybir.MatmulPerfMode.DoubleRow
            nch = KT // 2
            last = 3 * nch - 1

            def qkv_matmul(ps_, wth_, wtl_, xmh, xml):
                # 3-pass hi/lo: xh*wh + xl*wh + xh*wl, 6 chunks of 256
                idx = 0
                for xt_, wt_ in ((xmh, wth_), (xml, wth_), (xmh, wtl_)):
                    for kk in range(nch):
                        nc.tensor.matmul(
                            ps_[:],
                            xt_[:, 2 * kk : 2 * kk + 2, :],
                            wt_[:, 2 * kk : 2 * kk + 2, :],
                            start=(idx == 0), stop=(idx == last),
                            perf_mode=DR,
                        )
                        idx += 1

            rqs = {}  # block -> [(rq_tile, dst), ...] pending transposes

            def stage_a(m):
                psq = psA.tile([128, NHC, HD], F32, tag="psq", bufs=cfg["psq"])
                psk = psA.tile([128, NHC, HD], F32, tag="psk", bufs=cfg["psk"])
                psv = psA.tile(
                    [128, NHC, HD], F32, tag="psv",
                    bufs=2 if cfg["psv2"] else 1,
                )
                xmh = xp.tile([128, KT, 128], F8, tag="xmh")
                xml = xp.tile([128, KT, 128], F8, tag="xml")
                nc.sync.dma_start(xmh[:], x8hr[:, :, m * 128 : (m + 1) * 128])
                nc.sync.dma_start(xml[:], x8lr[:, :, m * 128 : (m + 1) * 128])
                # v first: its PSUM drain (Act copies) overlaps the q/k matmuls
                qkv_matmul(psv, wvth, wvtl, xmh, xml)
                if cfg["vdve"]:
                    nc.vector.tensor_copy(vext[:, :, m, 0:128], psv[:])
                else:
                    nc.scalar.copy(vext[:, :, m, 0:128], psv[:])
                qkv_matmul(psq, wqth, wqtl, xmh, xml)
                qkv_matmul(psk, wkth, wktl, xmh, xml)

                cosm = cost[:, m]  # [128, 3, 64]
                sinm = sint[:, m]
                # mean-of-squares from the (pre-rope) projections via Act
                # Square+accum (rope preserves per-token norms), then rsqrt
                # via Newton on Pool: y <- y*(1.5 - 0.5 v y^2)
                sqs = rp.tile([128, 8], F32, tag="sqs")
                scr = rp.tile([128, NHC, HD], F32, tag="scr")
                for si, src in ((0, psq), (4, psk)):
                    for hh in range(NHC):
                        nc.scalar.activation(
                            scr[:, hh, :], src[:, hh, :], AF.Square,
                            scale=SQ_SCALE,
                            accum_out=sqs[:, si + hh : si + hh + 1],
                        )
                ny = rp.tile([128, 8], F32, tag="ny", bufs=4)
                nt = rp.tile([128, 8], F32, tag="nt")
                nz = rp.tile([128, 8], F32, tag="nz")
                ne = nc.gpsimd
                # seed: minimax-ish secant of 1/sqrt(v) on v in [0.5, 1.8]
                # (v = mean(q^2) concentrates near 1); |eps0| <= ~16% even at
                # v=2, and each Newton step cubes the error (~1.5 eps^2)
                ne.tensor_scalar(ny[:], sqs[:], -0.5, 1.591, MUL, ADD)
                for _ in range(cfg["newti"]):
                    ne.tensor_mul(nt[:], ny[:], ny[:])
                    ne.tensor_mul(nt[:], nt[:], sqs[:])
                    ne.tensor_scalar(nz[:], nt[:], -0.5, 1.5, MUL, ADD)
                    ne.tensor_mul(ny[:], ny[:], nz[:])
                aeng = nc.gpsimd if cfg["padd"] else nc.vector
                reng = nc.gpsimd if cfg["rqpool"] else nc.vector
                rqs[m] = []
                for src, si, cc, dst in ((psq, 0, C_Q, qt), (psk, 4, C_K, kt)):
                    t1 = rp.tile([128, NHC, 64], F32, tag="t1")
                    t2 = rp.tile([128, NHC, 64], F32, tag="t2")
                    t3 = rp.tile([128, NHC, 64], F32, tag="t3")
                    t4 = rp.tile([128, NHC, 64], F32, tag="t4")
                    u = rp.tile([128, NHC, HD], F32, tag="u")
                    nc.vector.tensor_mul(t1[:], src[:, :, 0:64], cosm)
                    nc.vector.tensor_mul(t2[:], src[:, :, 64:128], sinm)
                    aeng.tensor_add(u[:, :, 0:64], t1[:], t2[:])
                    nc.vector.tensor_mul(t3[:], src[:, :, 64:128], cosm)
                    nc.vector.tensor_mul(t4[:], src[:, :, 0:64], sinm)
                    aeng.tensor_sub(u[:, :, 64:128], t3[:], t4[:])
                    rq = rp.tile([128, NHC, HD], F16, tag="rq", bufs=6)
                    for hh in range(NHC):
                        reng.tensor_scalar(
                            rq[:, hh, :], u[:, hh, :],
                            ny[:, si + hh : si + hh + 1], cc, MUL, MUL,
                        )
                    rqs[m].append((rq, dst))

            def stage_tp(m):
                # PE-transpose block m's q/k to [d, t] — deferred one block so
                # the squares->Newton->rq chain hides behind the next block's
                # projection matmuls instead of stalling the in-order PE
                for rq, dst in rqs.pop(m):
                    tp = tpps.tile([128, NHC, 128], F16, tag="tp")
                    for hh in range(NHC):
                        nc.tensor.transpose(tp[:, hh, :], rq[:, hh, :], idt[:])
                    nc.vector.tensor_copy(dst[:, :, m * 128 : (m + 1) * 128], tp[:])

            headpos_by_pr = {}

            def att_scores(pr):
                i0, i1 = 2 * pr, 2 * pr + 1
                w0, w1 = set(_window(i0)), set(_window(i1))
                js = sorted(w0 | w1)
                # pack (j, half) entries into <=512-col score tiles; a key
                # block j only gets the query columns whose window contains it
                entries = []  # (j, [halves])
                for j in js:
                    hs = [h for h, w in ((0, w0), (1, w1)) if j in w]
                    entries.append((j, hs))
                tiles = []  # (ents=[(j, h, off)], width)
                cur, off = [], 0
                for j, hs in entries:
                    w = 128 * len(hs)
                    if off + w > 512:
                        tiles.append((cur, off))
                        cur, off = [], 0
                    for h in hs:
                        cur.append((j, h, off))
                        off += 128
                if cur:
                    tiles.append((cur, off))

                headpos = []
                for hh in range(NHC):
                    pos = {}
                    for ents, width in tiles:
                        sps = spsum.tile([128, 512], F32, tag="sps")
                        ii = 0
                        while ii < len(ents):
                            j = ents[ii][0]
                            hs = [e[1] for e in ents if e[0] == j]
                            o = ents[ii][2]
                            w = 128 * len(hs)
                            q0 = (i0 + hs[0]) * 128
                            nc.tensor.matmul(
                                sps[:, o : o + w],
                                kt[:, hh, j * 128 : (j + 1) * 128],
                                qt[:, hh, q0 : q0 + w],
                                start=True, stop=True,
                            )
                            ii += len(hs)
                        ptile = pp.tile([128, 512], F16, tag="pt")
                        nc.scalar.activation(
                            ptile[:, 0:width], sps[:, 0:width], AF.Exp,
                            bias=b_shift, scale=1.0,
                        )
                        for j, h, o in ents:
                            pos[(j, h)] = (ptile, o)

                    def mask_mult(j, h, mask):
                        t, o = pos[(j, h)]
                        nc.gpsimd.tensor_mul(
                            t[:, o : o + 128], t[:, o : o + 128], mask[:]
                        )

                    mask_mult(i0, 0, mdg)
                    mask_mult(i1, 1, mdg)
                    if i0 >= WB:
                        mask_mult(i0 - WB, 0, medg)
                    if i1 >= WB:
                        mask_mult(i1 - WB, 1, medg)
                    headpos.append(pos)
                headpos_by_pr[pr] = headpos

            def att_pv(pr):
                i0, i1 = 2 * pr, 2 * pr + 1
                headpos = headpos_by_pr.pop(pr)
                # both p@v matmul groups first; the DVE normalize of block i0
                # runs while PE does block i1's matmuls, then the transposes
                ysbs = []
                for half, i in enumerate((i0, i1)):
                    jsi = _window(i)
                    ops = spsum.tile([128, NHC, 132], F32, tag="sps")
                    for hh in range(NHC):
                        pos = headpos[hh]
                        for idx, j in enumerate(jsi):
                            t, o = pos[(j, half)]
                            nc.tensor.matmul(
                                ops[:, hh, 0:129],
                                t[:, o : o + 128],
                                vext[:, hh, j, 0:129],
                                start=(idx == 0), stop=(idx == len(jsi) - 1),
                            )
                    rden = yp.tile([128, 4], F32, tag="rden")
                    nc.vector.reciprocal(rden[:, 0:3], ops[:, :, 128])
                    ysb = yp.tile([128, NHC, 128], F16, tag="ysb")
                    for hh in range(NHC):
                        nc.vector.tensor_scalar_mul(
                            ysb[:, hh, :], ops[:, hh, 0:128], rden[:, hh : hh + 1]
                        )
                    ysbs.append(ysb)
                for half, i in enumerate((i0, i1)):
                    ytp = tpps.tile([128, NHC, 128], F16, tag="tp")
                    for hh in range(NHC):
                        nc.tensor.transpose(ytp[:, hh, :], ysbs[half][:, hh, :], idt[:])
                    nc.vector.tensor_copy(yt[:, :, i * 128 : (i + 1) * 128], ytp[:])

            def out_proj(m):
                if not cfg["odma"]:
                    osb = op_sb.tile([128, HIDDEN], F16, tag="osb")
                for n in range(3):
                    po = spsum.tile([128, 512], F32, tag="sps")
                    for hh in range(NHC):
                        nc.tensor.matmul(
                            po[:],
                            yt[:, hh, m * 128 : (m + 1) * 128],
                            wpt[:, hh, n * 512 : (n + 1) * 512],
                            start=(hh == 0), stop=(hh == NHC - 1),
                        )
                    if cfg["odma"]:
                        nc.sync.dma_start(
                            outp.ap()[
                                m * 128 : (m + 1) * 128, n * 512 : (n + 1) * 512
                            ],
                            po[:],
                        )
                    elif cfg["osbdve"] or n % 2 == 0:
                        nc.vector.tensor_copy(osb[:, n * 512 : (n + 1) * 512], po[:])
                    else:
                        nc.scalar.copy(osb[:, n * 512 : (n + 1) * 512], po[:])
                if not cfg["odma"]:
                    nc.sync.dma_start(outp.ap()[m * 128 : (m + 1) * 128, :], osb[:])

            # fused pair-major schedule, software-pipelined: transposes lag
            # their projection by one block; attention lags transposes by one
            # pair; out-proj lags attention by one pair and is split around
            # p@v so exp/mask latency hides behind out-proj matmuls
            nreps = cfg.get("reps", 0)
            repctx = tc.For_i(0, nreps, 1) if nreps else None
            if repctx is not None:
                repctx.__enter__()
            if cfg.get("pipe", 0):
                stage_a(0)
                stage_a(1)
                stage_tp(0)
                for pr in range(1, NB // 2):
                    stage_a(2 * pr)
                    stage_tp(2 * pr - 1)
                    stage_a(2 * pr + 1)
                    stage_tp(2 * pr)
                    att_scores(pr - 1)
                    if pr >= 2:
                        out_proj(2 * pr - 4)
                    att_pv(pr - 1)
                    if pr >= 2:
                        out_proj(2 * pr - 3)
                stage_tp(NB - 1)
                att_scores(NB // 2 - 1)
                out_proj(NB - 4)
                att_pv(NB // 2 - 1)
                out_proj(NB - 3)
                out_proj(NB - 2)
                out_proj(NB - 1)
            else:
                for pr in range(NB // 2):
                    stage_a(2 * pr)
                    stage_tp(2 * pr)
                    stage_a(2 * pr + 1)
                    stage_tp(2 * pr + 1)
                    att_scores(pr)
                    att_pv(pr)
                    out_proj(2 * pr)
                    out_proj(2 * pr + 1)
            if repctx is not None:
                repctx.__exit__(None, None, None)

    nc.compile()
    return nc


def _get_nc():
    global _cached_nc
    if _cached_nc is None:
        _cached_nc = _build()
    return _cached_nc


def kernel(x, cos, sin, Wq, Wk, Wv, Wp, window, _trace=False, _result_holder=None):
    x = np.asarray(x, dtype=np.float32)
    cos = np.asarray(cos, dtype=np.float32)
    sin = np.asarray(sin, dtype=np.float32)
    Wq = np.asarray(Wq, dtype=np.float32)
    Wk = np.asarray(Wk, dtype=np.float32)
    Wv = np.asarray(Wv, dtype=np.float32)
    Wp = np.asarray(Wp, dtype=np.float32)
    assert int(window) == 1024, f"kernel hardcodes window=1024, got {window}"

    cosn = np.ascontiguousarray(cos[0, :, 0, :])  # [T, 64]
    sinn = np.ascontiguousarray(sin[0, :, 0, :])
    cos3 = np.tile(cosn, (1, NHC)).astype(np.float32)  # [T, 192]
    sin3 = np.tile(sinn, (1, NHC)).astype(np.float32)

    c = np.arange(128)[:, None]
    r = np.arange(128)[None, :]
    mdiag = (c <= r).astype(np.float16)
    medge = (r <= c).astype(np.float16)
    ident = np.eye(128, dtype=np.float16)

    import ml_dtypes

    F8NP = ml_dtypes.float8_e4m3fn

    def hilo(a):
        a = np.ascontiguousarray(a).astype(np.float32)
        hi = a.astype(F8NP)
        lo = (a - hi.astype(np.float32)).astype(F8NP)
        return hi, lo

    xsp = {b: hilo(x[b].T) for b in range(B)}
    wsp = {}
    for g in range(4):
        S = slice(g * NHC * HD, (g + 1) * NHC * HD)
        wsp[g] = (
            hilo(Wq[S, :].T * 64.0),
            hilo(Wk[S, :].T * 64.0),
            hilo(Wv[S, :].T * 64.0),
        )

    in_maps = []
    for core in range(8):
        b = core // 4
        g = core % 4
        S = slice(g * NHC * HD, (g + 1) * NHC * HD)
        (xh, xl), ((wqh, wql), (wkh, wkl), (wvh, wvl)) = xsp[b], wsp[g]
        m = {
            "cos3": cos3,
            "sin3": sin3,
            "mdiag": mdiag,
            "medge": medge,
            "ident": ident,
            "x8h": xh, "x8l": xl, "wq8h": wqh, "wq8l": wql,
            "wk8h": wkh, "wk8l": wkl, "wv8h": wvh, "wv8l": wvl,
            "wp16": np.ascontiguousarray(Wp[:, S].T / 64.0).astype(np.float16),
        }
        in_maps.append(m)

    nc = _get_nc()
    res = run_bass_kernel_spmd(nc, in_maps, list(range(8)), trace=_trace)
    if _result_holder is not None:
        _result_holder.append(res)

    out = np.zeros((B, T, HIDDEN), dtype=np.float32)
    for core in range(8):
        out[core // 4] += np.asarray(res.results[core]["outp"], dtype=np.float32)
    return out
